# revision 14
# baseline (speedup 1.0000x reference)
"""Trainium2 Bass kernel for nn_EnhancedGCN42 (4-layer GCN + MLP classifier).

Strategy (8 NeuronCores, SPMD single NEFF):
  - Nodes dst-sharded: device d owns dst nodes [d*12500, (d+1)*12500).
  - A-hat = D^-1/2 (A+I) D^-1/2 factorized: tables store dis*h rows (bf16);
    dis_dst applied per dst-tile via a replicated disrow operand fused into
    the PSUM eviction multiply.
  - Self-loop edges included in the gather lists (no separate identity path).
  - Per layer: per-edge rows gathered via dma_gather (4 SWDGE queues),
    aggregated per 128-dst tile by matmuls with lhsT=gathered rows (K=slots),
    rhs=S (one-hot, fp8) -> transposed PSUM [feat, dst].
  - S matrices are graph-constant: built once during phase 1 (group-wise
    is_equal on DVE + Square/Relu on ACT), cached to DRAM as fp8, and
    DMA-loaded in phases 2-4 (sequential, cheap) instead of rebuilt.
  - Epilogue fully in transposed [feat, dst] layout: bf16 dense W matmuls,
    folded BN scale/bias on ACT; one PE transpose before each table write.
  - Tables split in 4 quarter tensors matching the 4 int16 gather ranges;
    AllGather per quarter fires as soon as its 25 dst-tiles are done, so
    next-phase gathers overlap the current phase tail.

kernel(**inputs) -> [100000, 2] float32.
"""
import hashlib
import numpy as np
import ml_dtypes

import concourse.bacc as bacc
import concourse.mybir as mybir
import concourse.tile as tile
from concourse.masks import make_identity
from concourse.bass_utils import run_bass_kernel_spmd

f32 = mybir.dt.float32
bf16 = mybir.dt.bfloat16
fp8 = mybir.dt.float8e4
i16 = mybir.dt.int16
i32 = mybir.dt.int32
nbf16 = ml_dtypes.bfloat16

P = 128
NDEV = 8
NQ = 4           # src index ranges == table quarters (int16 limit)
EPS = 1e-5
WTAB = 128       # table row = 128 cols bf16 = 256B
BLK = 8
ACT_EVERY = 3    # every ACT_EVERY-th S group built on the scalar engine


def _prep(x, edge_index, params, N):
    """Host preprocessing: graph partition + folded constants."""
    SHARD = N // NDEV                      # 12500
    NT = 100                               # dst tiles per device (padded)
    TSHARD = NT * P                        # 12800
    QSH = TSHARD // NQ                     # 3200 rows per quarter-shard
    NTQ = NT // NQ                         # 25 tiles per quarter
    TROWS = TSHARD * NDEV                  # 102400
    RNGW = TROWS // NQ                     # 25600 (= NDEV*QSH), int16-safe
    assert RNGW <= 32768 and RNGW % P == 0

    ei = edge_index.astype(np.int64)
    loop = np.arange(N, dtype=np.int64)
    src = np.concatenate([ei[0], loop])
    dst = np.concatenate([ei[1], loop])
    deg = np.bincount(dst, minlength=N).astype(np.float32)
    dis = (1.0 / np.sqrt(deg)).astype(np.float32)

    HSH = TSHARD // 2
    HALF = TROWS // 2

    def rowmap(n):
        sh = n // SHARD
        i = n - sh * SHARD
        return np.where(i < HSH, sh * HSH + i, HALF + sh * HSH + (i - HSH))

    psrc = rowmap(src)

    counts = np.zeros((NDEV, NT, NQ), dtype=np.int64)
    dev_edges = []
    for d in range(NDEV):
        m = (dst >= d * SHARD) & (dst < (d + 1) * SHARD)
        es = psrc[m]
        el = dst[m] - d * SHARD
        t_id = el >> 7
        r_id = es // RNGW
        order = np.lexsort((es, r_id, t_id))  # (tile, range, src-ascending)
        es, el, t_id, r_id = es[order], el[order], t_id[order], r_id[order]
        np.add.at(counts[d], (t_id, r_id), 1)
        dev_edges.append((es, el))

    grp_rows = ((counts.max(axis=0) + 15) // 16 * 16).astype(np.int64)  # [NT, NQ]

    n_blk = (NT + BLK - 1) // BLK
    grp_off = np.zeros((NT, NQ), dtype=np.int64)
    blk_off = np.zeros((n_blk, NQ), dtype=np.int64)
    blk_rows = np.zeros((n_blk, NQ), dtype=np.int64)
    acc = 0
    for b in range(n_blk):
        for r in range(NQ):
            blk_off[b, r] = acc
            for t in range(b * BLK, min((b + 1) * BLK, NT)):
                grp_off[t, r] = acc
                acc += grp_rows[t, r]
            acc = (acc + P - 1) // P * P  # pad gather to whole chunks
            blk_rows[b, r] = acc - blk_off[b, r]
    TOT = acc

    # chunk-use enumeration: per (t, r) the 128-row chunks its group overlaps.
    # uses[t][r] = (chunk local to block, use column, global chunk)
    uses = [[[] for _ in range(NQ)] for _ in range(NT)]
    blk_du0 = np.zeros((n_blk, NQ), dtype=np.int64)   # first use col of (b, r)
    blk_nlu = np.zeros((n_blk, NQ), dtype=np.int64)   # n use cols of (b, r)
    n_uses = 0
    for b in range(n_blk):
        for r in range(NQ):
            blk_du0[b, r] = n_uses
            for t in range(b * BLK, min((b + 1) * BLK, NT)):
                g0, g1 = grp_off[t, r], grp_off[t, r] + grp_rows[t, r]
                c0, c1 = int(g0 // P), int((g1 + P - 1) // P)
                for ci in range(c0, c1):
                    uses[t][r].append((ci - int(blk_off[b, r]) // P, n_uses, ci))
                    n_uses += 1
            blk_nlu[b, r] = n_uses - blk_du0[b, r]

    idx_w = np.zeros((NDEV, P, TOT // 16), dtype=np.int16)
    dstl_w = np.full((NDEV, P, n_uses), 255.0, dtype=nbf16)
    for d in range(NDEV):
        es, el = dev_edges[d]
        IDX = np.zeros(TOT, dtype=np.int16)
        DLOC = np.full(TOT, 255.0, dtype=np.float32)
        OWNER = np.full(TOT, -1, dtype=np.int64)
        pos = 0
        for t in range(NT):
            for r in range(NQ):
                c = int(counts[d, t, r])
                o = int(grp_off[t, r])
                IDX[o:o + c] = (es[pos:pos + c] - r * RNGW).astype(np.int16)
                DLOC[o:o + c] = (el[pos:pos + c] - t * P).astype(np.float32)
                OWNER[o:o + int(grp_rows[t, r])] = t
                pos += c
        idx_w[d] = np.tile(IDX.reshape(-1, 16).T, (8, 1))
        for t in range(NT):
            for r in range(NQ):
                for (_lc, du, ci) in uses[t][r]:
                    rows = np.arange(ci * P, (ci + 1) * P)
                    dstl_w[d][:, du] = np.where(OWNER[rows] == t, DLOC[rows], 255.0)

    dis_pad = np.zeros(NDEV * TSHARD, dtype=np.float32)
    for s in range(NDEV):
        dis_pad[s * TSHARD:s * TSHARD + SHARD] = dis[s * SHARD:(s + 1) * SHARD]
    dis_t = np.stack([
        dis_pad[d * TSHARD:(d + 1) * TSHARD].reshape(NT, P).T for d in range(NDEV)
    ])  # [NDEV, P, NT]
    # dis replicated on all 128 partitions, laid out [P, NT*128]
    disrow = np.ascontiguousarray(
        np.broadcast_to(dis_pad.reshape(NDEV, 1, TSHARD), (NDEV, P, TSHARD))
    ).astype(nbf16)  # [NDEV, P, NT*128]

    # x-tilde table (bf16, padded cols), quarter-interleaved row space
    xt = np.zeros((TROWS, WTAB), dtype=nbf16)
    v = (dis[:, None] * x).astype(nbf16)
    rows = rowmap(np.arange(N))
    xt[rows, :x.shape[1]] = v

    def fold(g, be, rm, rv, b):
        k = (1.0 / np.sqrt(rv + EPS)).astype(np.float32)
        s = g * k
        t = (b - rm) * s + be
        return s.astype(np.float32), t.astype(np.float32)

    s1, t1 = fold(params["g1"], params["be1"], params["rm1"], params["rv1"], params["b1"])
    s2, t2 = fold(params["g2"], params["be2"], params["rm2"], params["rv2"], params["b2"])
    s3, t3 = fold(params["g3"], params["be3"], params["rm3"], params["rv3"], params["b3"])
    s4, t4 = fold(params["g4"], params["be4"], params["rm4"], params["rv4"], params["b4"])
    zk = (1.0 / np.sqrt(params["crv1"] + EPS)).astype(np.float32)
    cs1 = params["cg1"] * zk
    ct1 = -params["crm1"] * cs1 + params["cbe1"]
    zk = (1.0 / np.sqrt(params["crv2"] + EPS)).astype(np.float32)
    cs2 = params["cg2"] * zk
    ct2 = -params["crm2"] * cs2 + params["cbe2"]
    cW2p = (cs1[:, None] * params["cW2"]).astype(np.float32)
    cb2p = (ct1 @ params["cW2"] + params["cb2"]).astype(np.float32)
    cW3p = (cs2[:, None] * params["cW3"]).astype(np.float32)
    cb3p = (ct2 @ params["cW3"] + params["cb3"]).astype(np.float32)

    vecs = np.zeros((P, 13), dtype=np.float32)
    vecs[:, 0], vecs[:, 1] = s1, t1
    vecs[:, 2], vecs[:, 3] = s2[:128], t2[:128]
    vecs[:, 4], vecs[:, 5] = s2[128:], t2[128:]
    vecs[:, 6], vecs[:, 7] = s3, t3
    vecs[:64, 8], vecs[:64, 9] = s4, t4
    vecs[:64, 10] = params["cb1"]
    vecs[:32, 11] = cb2p
    vecs[:2, 12] = cb3p

    W3 = params["W3"]  # [256, 128]
    W3pk = np.concatenate([W3[:128], W3[128:]], axis=1)  # [128, 256] K-halves

    return dict(
        N=N, SHARD=SHARD, TSHARD=TSHARD, NT=NT, TROWS=TROWS, RNGW=RNGW,
        QSH=QSH, NTQ=NTQ,
        TOT=TOT, uses=uses, n_uses=n_uses,
        n_blk=n_blk, blk_off=blk_off, blk_rows=blk_rows,
        blk_du0=blk_du0, blk_nlu=blk_nlu,
        idx_w=idx_w, dstl_w=dstl_w, dis_t=dis_t, disrow=disrow, xt=xt,
        vecs=vecs,
        W1=params["W1"].astype(nbf16), W2=params["W2"].astype(nbf16),
        W3=W3pk.astype(nbf16), W4=params["W4"].astype(nbf16),
        cW1=params["cW1"].astype(nbf16), cW2p=cW2p.astype(nbf16),
        cW3p=cW3p.astype(nbf16),
        d_in=x.shape[1],
    )


def _build(meta):
    """Build the Bass program (same for all cores)."""
    NT, RNGW, QSH, NTQ = meta["NT"], meta["RNGW"], meta["QSH"], meta["NTQ"]
    TOT = meta["TOT"]
    uses, n_uses = meta["uses"], meta["n_uses"]
    n_blk, blk_off, blk_rows = meta["n_blk"], meta["blk_off"], meta["blk_rows"]
    blk_du0, blk_nlu = meta["blk_du0"], meta["blk_nlu"]
    D_IN = meta["d_in"]
    NLU_MAX = int(blk_nlu.max())
    KMAX = max((len(uses[t][r]) for t in range(NT) for r in range(NQ)),
               default=1)

    nc = bacc.Bacc(None, target_bir_lowering=False, num_swdge_queues=4)
    t_xt = nc.dram_tensor("xt", [NDEV * NT * P, WTAB], bf16, kind="ExternalInput")
    t_idx = nc.dram_tensor("idx", [P, TOT // 16], i16, kind="ExternalInput")
    t_dstl = nc.dram_tensor("dstl", [P, n_uses], bf16, kind="ExternalInput")
    t_dis = nc.dram_tensor("dis", [P, NT], f32, kind="ExternalInput")
    t_disrow = nc.dram_tensor("disrow", [P, NT * P], bf16, kind="ExternalInput")
    t_vecs = nc.dram_tensor("vecs", [P, 13], f32, kind="ExternalInput")
    t_W1 = nc.dram_tensor("W1", [D_IN, 128], bf16, kind="ExternalInput")
    t_W2 = nc.dram_tensor("W2", [128, 256], bf16, kind="ExternalInput")
    t_W3 = nc.dram_tensor("W3", [128, 256], bf16, kind="ExternalInput")  # K-halves
    t_W4 = nc.dram_tensor("W4", [128, 64], bf16, kind="ExternalInput")
    t_cW1 = nc.dram_tensor("cW1", [64, 64], bf16, kind="ExternalInput")
    t_cW2 = nc.dram_tensor("cW2p", [64, 32], bf16, kind="ExternalInput")
    t_cW3 = nc.dram_tensor("cW3p", [32, 2], bf16, kind="ExternalInput")
    t_out = nc.dram_tensor("outT", [2, NT * P], f32, kind="ExternalOutput")
    t_scache = nc.dram_tensor("scache", [P, n_uses, P], fp8)

    # per-phase half tables + per-half collective inputs (big AGs = fast AGs)
    HSH = NT * P // 2
    cc_in = [[nc.dram_tensor(f"cc_in{k}_{j}", [HSH, WTAB], bf16)
              for j in range(2)] for k in range(3)]
    tabh = [[nc.dram_tensor(f"tab{k}_{j}", [NDEV * HSH, WTAB], bf16,
                            addr_space="Shared")
             for j in range(2)] for k in range(3)]

    with tile.TileContext(nc) as tc:
        with (
            tc.tile_pool(name="const", bufs=1) as cpool,
            tc.tile_pool(name="gp", bufs=6) as gpool,
            tc.tile_pool(name="sp", bufs=6) as spool,
            tc.tile_pool(name="yq", bufs=3) as yqpool,
            tc.tile_pool(name="pagg", bufs=2, space="PSUM") as pagg,
            tc.tile_pool(name="paux", bufs=3, space="PSUM") as paux,
            tc.tile_pool(name="pacc", bufs=2, space="PSUM") as pacc,
            tc.tile_pool(name="ep", bufs=3) as ep,
        ):
            # ---- constants
            idx_sb = cpool.tile([P, TOT // 16], i16)
            nc.sync.dma_start(out=idx_sb[:], in_=t_idx[:])
            dstl_sb = cpool.tile([P, n_uses], bf16)
            nc.sync.dma_start(out=dstl_sb[:], in_=t_dstl[:])
            dstln_sb = cpool.tile([P, n_uses], f32)
            nc.vector.tensor_scalar_mul(dstln_sb[:], dstl_sb[:], -1.0)
            dis_sb = cpool.tile([P, NT], f32)
            nc.sync.dma_start(out=dis_sb[:], in_=t_dis[:])
            disrow_sb = cpool.tile([P, NT * P], bf16)
            nc.sync.dma_start(out=disrow_sb[:], in_=t_disrow[:])
            vecs_sb = cpool.tile([P, 13], f32)
            nc.sync.dma_start(out=vecs_sb[:], in_=t_vecs[:])
            W1_sb = cpool.tile([D_IN, 128], bf16)
            nc.sync.dma_start(out=W1_sb[:], in_=t_W1[:])
            W2_sb = cpool.tile([128, 256], bf16)
            nc.sync.dma_start(out=W2_sb[:], in_=t_W2[:])
            W3_sb = cpool.tile([128, 256], bf16)
            nc.sync.dma_start(out=W3_sb[:], in_=t_W3[:])
            W4_sb = cpool.tile([128, 64], bf16)
            nc.sync.dma_start(out=W4_sb[:], in_=t_W4[:])
            cW1_sb = cpool.tile([64, 64], bf16)
            nc.sync.dma_start(out=cW1_sb[:], in_=t_cW1[:])
            cW2_sb = cpool.tile([64, 32], bf16)
            nc.sync.dma_start(out=cW2_sb[:], in_=t_cW2[:])
            cW3_sb = cpool.tile([32, 2], bf16)
            nc.sync.dma_start(out=cW3_sb[:], in_=t_cW3[:])
            ident = cpool.tile([P, P], f32)
            make_identity(nc, ident[:])
            iota_i = cpool.tile([P, KMAX, P], i32)
            nc.gpsimd.iota(iota_i[:], pattern=[[0, KMAX], [1, P]], base=0,
                           channel_multiplier=0)
            iota_bf = cpool.tile([P, KMAX, P], bf16)
            nc.vector.tensor_copy(out=iota_bf[:], in_=iota_i[:])

            AluEq = mybir.AluOpType.is_equal
            ACTF = mybir.ActivationFunctionType
            sctr = [0]

            def build_s_group(sblk, u_lo, du0, length):
                """Build S cols [u_lo, u_lo+length) of block tile sblk
                (one-hot vs dstl) on DVE or ACT (round-robin)."""
                sctr[0] += 1
                if sctr[0] % ACT_EVERY != 0:
                    nc.vector.tensor_tensor(
                        out=sblk[:, u_lo:u_lo + length, :],
                        in0=dstl_sb[:, du0:du0 + length].to_broadcast(
                            [P, length, P]),
                        in1=iota_bf[:, :length, :],
                        op=AluEq,
                    )
                else:
                    yq = yqpool.tile([P, KMAX, P], bf16, tag="yq")
                    for ui in range(length):
                        nc.scalar.activation(
                            yq[:, ui, :], iota_bf[:, ui, :], ACTF.Square,
                            bias=dstln_sb[:, du0 + ui:du0 + ui + 1])
                    nc.scalar.activation(sblk[:, u_lo:u_lo + length, :],
                                         yq[:, :length, :], ACTF.Relu,
                                         bias=1.0, scale=-1.0)

            def phase(k, tables, wf, epilogue, cc_out):
                """k: phase index (0 builds S, 1-3 load); tables: 4 per-range
                DRAM APs; wf: feature cols used; cc_out: (cc_in[k], tabq[k])
                or None."""
                for b in range(n_blk):
                    tiles = range(b * BLK, min((b + 1) * BLK, NT))
                    gt = {}
                    st = {}
                    for r in range(NQ):
                        rows = int(blk_rows[b, r])
                        nlu = int(blk_nlu[b, r])
                        du0 = int(blk_du0[b, r])
                        if rows == 0:
                            continue
                        g = gpool.tile([P, rows // P, WTAB], bf16, tag="g")
                        off = int(blk_off[b, r])
                        nc.gpsimd.dma_gather(
                            out_ap=g[:],
                            in_ap=tables[r],
                            idxs_ap=idx_sb[:, off // 16:(off + rows) // 16],
                            num_idxs=rows,
                            num_idxs_reg=rows,
                            elem_size=WTAB,
                            single_packet=False,
                            queue_num=r,
                        )
                        gt[r] = g
                        sblk = spool.tile([P, nlu, P], fp8, tag="s")
                        if k == 0:
                            for t in tiles:
                                ul = uses[t][r]
                                if ul:
                                    build_s_group(sblk, ul[0][1] - du0,
                                                  ul[0][1], len(ul))
                            nc.sync.dma_start(
                                out=t_scache[:, du0:du0 + nlu, :],
                                in_=sblk[:])
                        else:
                            nc.sync.dma_start(
                                out=sblk[:],
                                in_=t_scache[:, du0:du0 + nlu, :])
                        st[r] = sblk
                    for t in tiles:
                        nmm = sum(len(uses[t][r]) for r in range(NQ))
                        if nmm > 0:
                            ps = pagg.tile([wf, P], f32, tag="pagg")
                            kk = 0
                            for r in range(NQ):
                                du0 = int(blk_du0[b, r])
                                for (lc, du, _ci) in uses[t][r]:
                                    nc.tensor.matmul(
                                        ps[:], lhsT=gt[r][:, lc, :wf],
                                        rhs=st[r][:, du - du0, :],
                                        start=(kk == 0), stop=(kk == nmm - 1),
                                    )
                                    kk += 1
                            epilogue(t, ps)
                        if cc_out is not None and (t + 1) % (NT // 2) == 0:
                            j = t // (NT // 2)
                            nc.gpsimd.collective_compute(
                                "AllGather", mybir.AluOpType.bypass,
                                replica_groups=[list(range(NDEV))],
                                ins=[cc_out[0][j][:]],
                                outs=[cc_out[1][j][:]],
                            )

            def evict(t, ps, wf, dt):
                """z[feat, dst] = ps * dis_dst (fused PSUM eviction)."""
                z = ep.tile([wf, P], dt, tag=f"z{wf}_{dt}")
                nc.vector.tensor_tensor(
                    out=z[:], in0=ps[:],
                    in1=disrow_sb[:wf, t * P:(t + 1) * P], op=mybir.AluOpType.mult)
                return z

            def write_tab(t, hT, w, k):
                """Transpose hT [w<=128, P] f32, scale by dis, write bf16
                node-major rows to cc quarter."""
                tp = paux.tile([P, w], f32, tag="mm")
                nc.tensor.transpose(tp[:], hT[:], ident[:w, :w])
                hb = ep.tile([P, w], bf16, tag=f"hb{k}")
                nc.scalar.activation(hb[:], tp[:], ACTF.Copy,
                                     scale=dis_sb[:, t:t + 1])
                j, tl = t // (NT // 2), t % (NT // 2)
                nc.sync.dma_start(
                    out=cc_in[k][j][tl * P:(tl + 1) * P, :w], in_=hb[:])

            # ================= Phase 1: L1 =================
            def ep1(t, ps):
                z = evict(t, ps, D_IN, bf16)
                hps = paux.tile([128, P], f32, tag="mm")
                nc.tensor.matmul(hps[:], lhsT=W1_sb[:], rhs=z[:],
                                 start=True, stop=True)
                hT = ep.tile([128, P], f32, tag="h1T")
                nc.scalar.activation(hT[:], hps[:], ACTF.Relu,
                                     bias=vecs_sb[:, 1:2], scale=vecs_sb[:, 0:1])
                write_tab(t, hT, 128, 0)

            def tab_rng(k):
                return [tabh[k][r // 2][(r % 2) * RNGW:(r % 2 + 1) * RNGW, :]
                        for r in range(NQ)]

            phase(0, [t_xt[r * RNGW:(r + 1) * RNGW, :] for r in range(NQ)],
                  D_IN, ep1, (cc_in[0], tabh[0]))

            # ================= Phase 2: L2 + dense L3 =================
            def ep2(t, ps):
                z = evict(t, ps, 128, bf16)
                y3ps = pacc.tile([128, P], f32, tag="acc")
                for h in range(2):
                    hps = paux.tile([128, P], f32, tag="mm")
                    nc.tensor.matmul(hps[:], lhsT=W2_sb[:, h * 128:(h + 1) * 128],
                                     rhs=z[:], start=True, stop=True)
                    h2T = ep.tile([128, P], bf16, tag="h2T")
                    nc.scalar.activation(h2T[:], hps[:], ACTF.Relu,
                                         bias=vecs_sb[:, 3 + 2 * h:4 + 2 * h],
                                         scale=vecs_sb[:, 2 + 2 * h:3 + 2 * h])
                    nc.tensor.matmul(y3ps[:], lhsT=W3_sb[:, h * 128:(h + 1) * 128],
                                     rhs=h2T[:], start=(h == 0), stop=(h == 1))
                y3T = ep.tile([128, P], f32, tag="y3T")
                nc.vector.tensor_copy(out=y3T[:], in_=y3ps[:])
                write_tab(t, y3T, 128, 1)

            phase(1, tab_rng(0), 128, ep2, (cc_in[1], tabh[1]))

            # ================= Phase 3: L3 agg + dense L4 =================
            def ep3(t, ps):
                z = evict(t, ps, 128, bf16)
                h3T = ep.tile([128, P], bf16, tag="h3T")
                nc.scalar.activation(h3T[:], z[:], ACTF.Relu,
                                     bias=vecs_sb[:, 7:8], scale=vecs_sb[:, 6:7])
                y4ps = paux.tile([64, P], f32, tag="mm")
                nc.tensor.matmul(y4ps[:], lhsT=W4_sb[:], rhs=h3T[:],
                                 start=True, stop=True)
                y4T = ep.tile([64, P], f32, tag="y4T")
                nc.vector.tensor_copy(out=y4T[:], in_=y4ps[:])
                write_tab(t, y4T, 64, 2)

            phase(2, tab_rng(1), 128, ep3, (cc_in[2], tabh[2]))

            # ================= Phase 4: L4 agg + classifier =================
            def ep4(t, ps):
                z = evict(t, ps, 64, bf16)
                h4T = ep.tile([64, P], bf16, tag="h4T")
                nc.scalar.activation(h4T[:], z[:], ACTF.Relu,
                                     bias=vecs_sb[:64, 9:10],
                                     scale=vecs_sb[:64, 8:9])
                u1ps = paux.tile([64, P], f32, tag="mm")
                nc.tensor.matmul(u1ps[:], lhsT=cW1_sb[:], rhs=h4T[:],
                                 start=True, stop=True)
                u1T = ep.tile([64, P], bf16, tag="u1T")
                nc.scalar.activation(u1T[:], u1ps[:], ACTF.Relu,
                                     bias=vecs_sb[:64, 10:11])
                u2ps = paux.tile([32, P], f32, tag="mm")
                nc.tensor.matmul(u2ps[:], lhsT=cW2_sb[:], rhs=u1T[:],
                                 start=True, stop=True)
                u2T = ep.tile([32, P], bf16, tag="u2T")
                nc.scalar.activation(u2T[:], u2ps[:], ACTF.Relu,
                                     bias=vecs_sb[:32, 11:12])
                ops_ = paux.tile([2, P], f32, tag="mm")
                nc.tensor.matmul(ops_[:], lhsT=cW3_sb[:], rhs=u2T[:],
                                 start=True, stop=True)
                oT = ep.tile([2, P], f32, tag="oT")
                nc.scalar.activation(oT[:], ops_[:], ACTF.Identity,
                                     bias=vecs_sb[:2, 12:13])
                nc.sync.dma_start(out=t_out[:, t * P:(t + 1) * P], in_=oT[:])

            phase(3, tab_rng(2), 64, ep4, None)

    nc.finalize()
    return nc


_CACHE = {}


def kernel(**inputs):
    x = np.asarray(inputs["x"], dtype=np.float32)
    edge_index = np.asarray(inputs["edge_index"])
    N = x.shape[0]
    key = hashlib.sha256(edge_index.tobytes()).hexdigest()[:16] + f"_{N}_{x.shape[1]}"
    if key not in _CACHE:
        meta = _prep(x, edge_index, inputs, N)
        nc = _build(meta)
        _CACHE[key] = (meta, nc)
    else:
        meta, nc = _CACHE[key]
        meta = dict(meta)
        m2 = _prep(x, edge_index, inputs, N)
        meta.update({k: m2[k] for k in (
            "xt", "vecs", "W1", "W2", "W3", "W4", "cW1", "cW2p", "cW3p",
            "dis_t", "disrow")})

    in_maps = []
    for d in range(NDEV):
        in_maps.append({
            "xt": meta["xt"],
            "idx": meta["idx_w"][d],
            "dstl": meta["dstl_w"][d],
            "dis": meta["dis_t"][d],
            "disrow": meta["disrow"][d],
            "vecs": meta["vecs"],
            "W1": meta["W1"], "W2": meta["W2"], "W3": meta["W3"], "W4": meta["W4"],
            "cW1": meta["cW1"], "cW2p": meta["cW2p"], "cW3p": meta["cW3p"],
        })
    res = None
    for _attempt in range(4):
        try:
            res = run_bass_kernel_spmd(nc, in_maps, core_ids=list(range(NDEV)), trace=False)
            break
        except Exception:
            if _attempt == 3:
                raise
    assert res is not None

    SHARD = meta["SHARD"]
    out = np.empty((N, 2), dtype=np.float32)
    for d in range(NDEV):
        out[d * SHARD:(d + 1) * SHARD] = res.results[d]["outT"][:, :SHARD].T
    return out
